# revision 11
# baseline (speedup 1.0000x reference)
"""DBPNet Trainium2 kernel: 8-core data-parallel Bass/Tile implementation.

Scheme:
  - batch-major state [32, N]: row = chan*16 + s (16 samples/core)
  - complex matvecs are "state-stationary": lhsT = combo(state) chunks
    [128, 32] (bf16), rhs = replicated matrix stacks (bf16, SBUF-resident)
  - ADMM algebra folded on host:
      Ax  = P' zmu + h          P' = rho*c1*(AAH - AAH Minv AAH)
      s   = R zmu + r0          R  = rho*c1*(I - Minv AAH)
      h   = Hm arc              Hm = c1*(I - AAH Minv)
      r0  = Mn arc              Mn = -c1*Minv
      x   = c1*rc + A^H s
    z eliminated:  zmu' = y + (2f-1)diff,  uy' = (1-f)diff - y,
    diff = Ax + u - y,  f = min(1, eps*rsqrt(|diff|^2))
  - M=32 matvecs packed 4-wide into PSUM col groups (concurrent MMs)
  - CNN in (co*4+q, (b', l)) layout with block-diagonal weights (fp32r);
    BN batch stats exact across cores via AllReduce, stat sums fused into
    the PSUM->SBUF copies (scalar engine accum_out) + DVE squares
"""
import numpy as np

B, Nv, Nt, F = 128, 512, 2048, 32
NCORE, BS = 8, 16
ITERS, ADMM = 5, 3
BN_EPS = 1e-5


# ---------------------------------------------------------------- host prep
def _host_prep(inputs):
    import ml_dtypes
    bf16 = ml_dtypes.bfloat16

    A = np.ascontiguousarray(np.asarray(inputs['A'], np.float32))
    Ar, Ai = A[0], A[1]
    Ac = Ar.astype(np.float64) + 1j * Ai.astype(np.float64)
    AAH = Ac @ Ac.conj().T

    rhos = np.exp(np.asarray(inputs['log_rho'], np.float32)).astype(np.float32)
    epss = np.exp(np.asarray(inputs['log_eps'], np.float32)).astype(np.float32)

    def cstack(M):
        """complex [Nv,Nv] -> [128, 8, 512] layout for comboN matvecs."""
        M1 = np.concatenate([M.real.T, M.imag.T], 0).astype(np.float32)
        return M1.reshape(8, 128, 512).transpose(1, 0, 2).copy()

    I = np.eye(Nv)
    stk_list, rho_to_idx, iter_stk_idx = [], {}, []
    for r in rhos:
        key = float(r)
        if key not in rho_to_idx:
            c1 = 1.0 / (key + 1e-8)
            Minv = np.linalg.inv(AAH + key * I)
            MA = Minv @ AAH
            Pp = key * c1 * (AAH - AAH @ MA)
            R = key * c1 * (I - MA)
            Mn = -c1 * Minv
            Hm = c1 * (I - AAH @ Minv)
            stk = np.stack([cstack(Pp), cstack(R), cstack(Mn), cstack(Hm)],
                           axis=1)                      # [128, 4, 8, 512]
            stk_list.append(stk.astype(bf16))
            rho_to_idx[key] = len(stk_list) - 1
        iter_stk_idx.append(rho_to_idx[float(r)])

    A1 = np.concatenate([Ar, Ai], 0)                    # [1024, 2048]
    AB = A1.reshape(8, 128, 2048).transpose(1, 0, 2).astype(bf16)
    AT1 = np.concatenate([Ar.T, Ai.T], 0)               # [4096, 512]
    ATR = AT1.reshape(32, 128, 512).transpose(1, 0, 2).astype(bf16)

    w1 = np.asarray(inputs['conv1_w'], np.float32)
    w2 = np.asarray(inputs['conv2_w'], np.float32)
    wf = np.asarray(inputs['convf_w'], np.float32)
    W1 = np.zeros((128, 128), np.float32)
    for dl in range(3):
        for ci in range(2):
            for q in range(4):
                W1[dl * 8 + ci * 4 + q, np.arange(F) * 4 + q] = w1[:, ci, dl]
    W2 = np.zeros((3, 128, 128), np.float32)
    WFm = np.zeros((3, 128, 8), np.float32)
    for dl in range(3):
        for ci in range(F):
            for q in range(4):
                W2[dl, ci * 4 + q, np.arange(F) * 4 + q] = w2[:, ci, dl]
                WFm[dl, ci * 4 + q, np.arange(2) * 4 + q] = wf[:, ci, dl]

    onesel = np.zeros((128, 32), np.float32)
    selback = np.zeros((128, 128), np.float32)   # rows 0-31 used
    for co in range(32):
        for q in range(4):
            onesel[co * 4 + q, co] = 1.0
            selback[co, co * 4 + q] = 1.0
    ident32 = np.zeros((128, 32), np.float32)
    ident32[:32, :32] = np.eye(32, dtype=np.float32)
    ones11 = np.zeros((128, 1), np.float32)
    ones11[0, 0] = 1.0

    WTS = np.concatenate(
        [W1] + [W2[d] for d in range(3)] + [WFm[d] for d in range(3)]
        + [onesel, selback, ident32, ones11], axis=1)

    g1 = np.asarray(inputs['bn1_g'], np.float32)
    b1 = np.asarray(inputs['bn1_b'], np.float32)
    g2 = np.asarray(inputs['bn2_g'], np.float32)
    b2 = np.asarray(inputs['bn2_b'], np.float32)
    fb = np.asarray(inputs['convf_b'], np.float32)
    CF = np.zeros((128, 16), np.float32)
    CF[:32, 0] = g1
    CF[:32, 1] = b1
    CF[:32, 2] = g2
    CF[:32, 3] = b2
    CF[:, 6] = BN_EPS   # col 5 stays zero (zero-bias AP)
    for it in range(ITERS):
        c1 = 1.0 / (float(rhos[it]) + 1e-8)
        CF[:16, 8 + it] = fb[0] * c1
        CF[16:32, 8 + it] = fb[1] * c1

    y = np.asarray(inputs['y'], np.float32)
    ybm_cores, ylhsT_cores = [], []
    for c in range(NCORE):
        ys = y[c * BS:(c + 1) * BS]
        ybm = np.concatenate([ys[:, 0], ys[:, 1]], 0)    # [32, Nv]
        ybm_cores.append(np.ascontiguousarray(ybm))
        sT = ybm.T                                       # [Nv, 32]
        bot = np.concatenate([sT[:, 16:], -sT[:, :16]], 1)
        comboH = np.concatenate([sT, bot], 0)            # [2Nv, 32]
        ylhsT_cores.append(
            comboH.reshape(8, 128, 32).transpose(1, 0, 2).astype(bf16))

    return dict(AB=AB, ATR=ATR, stk_list=stk_list, iter_stk_idx=iter_stk_idx,
                rhos=rhos, epss=epss, WTS=WTS, CF=CF,
                ybm_cores=ybm_cores, ylhsT_cores=ylhsT_cores)


# WTS column offsets
W1_C = 0
W2_C = 128
WF_C = 128 + 384
OSEL_C = WF_C + 24
SELB_C = OSEL_C + 32
ID32_C = SELB_C + 128
ONE1_C = ID32_C + 32
WTS_W = ONE1_C + 1


# ---------------------------------------------------------------- program
def _build_program(prep):
    import concourse.bacc as bacc
    import concourse.tile as tile
    import concourse.mybir as mybir

    dt = mybir.dt
    f32, f32r, bf = dt.float32, dt.float32r, dt.bfloat16
    AX = mybir.AxisListType
    OP = mybir.AluOpType
    AF = mybir.ActivationFunctionType

    nu = len(prep['stk_list'])
    rhos, epss = prep['rhos'], prep['epss']
    cnt = float(B * Nt)

    nc = bacc.Bacc("TRN2", target_bir_lowering=False, debug=False,
                   num_devices=NCORE)

    AB_d = nc.dram_tensor("AB", [128, 8, 2048], bf, kind="ExternalInput")
    AT_d = nc.dram_tensor("ATR", [128, 32, 512], bf, kind="ExternalInput")
    STK_d = nc.dram_tensor("STKS", [nu, 128, 4, 8, 512], bf,
                           kind="ExternalInput")
    WTS_d = nc.dram_tensor("WTS", [128, WTS_W], f32r, kind="ExternalInput")
    CF_d = nc.dram_tensor("CF", [128, 16], f32, kind="ExternalInput")
    Y_d = nc.dram_tensor("YBM", [32, 512], f32r, kind="ExternalInput")
    YL_d = nc.dram_tensor("YL", [128, 8, 32], bf, kind="ExternalInput")
    XO_d = nc.dram_tensor("XOUT", [32, 2048], f32r, kind="ExternalOutput")

    with tile.TileContext(nc) as tc:
        with (
            tc.tile_pool(name="cst", bufs=1) as cst,
            tc.tile_pool(name="st", bufs=1) as stp,
            tc.tile_pool(name="cmb", bufs=1) as cmb,
            tc.tile_pool(name="act", bufs=5) as actp,
            tc.tile_pool(name="xin", bufs=2) as xinp,
            tc.tile_pool(name="sc", bufs=1) as scp,
            tc.tile_pool(name="psA", bufs=2, space="PSUM") as psA,
            tc.tile_pool(name="psB", bufs=3, space="PSUM") as psB,
            tc.tile_pool(name="psC", bufs=1, space="PSUM") as psC,
            tc.tile_pool(name="psD", bufs=2, space="PSUM") as psD,
            tc.tile_pool(name="ddr", bufs=2, space="DRAM") as ddr,
        ):
            # ---- constants into SBUF ----
            ab = cst.tile([128, 8, 2048], bf, tag="ab")
            at = cst.tile([128, 32, 512], bf, tag="at")
            stk = cst.tile([128, 4, 8, 512], bf, tag="stk")
            wts = cst.tile([128, WTS_W], f32r, tag="wts")
            cf = cst.tile([128, 16], f32, tag="cf")
            yl = cst.tile([128, 8, 32], bf, tag="yl")
            nc.sync.dma_start(ab[:], AB_d[:])
            nc.sync.dma_start(at[:], AT_d[:])
            nc.sync.dma_start(wts[:], WTS_d[:])
            nc.sync.dma_start(cf[:], CF_d[:])
            nc.sync.dma_start(yl[:], YL_d[:])
            if nu == 1:
                nc.sync.dma_start(stk[:], STK_d[0])

            W1 = wts[:, W1_C:W1_C + 128]
            W2 = [wts[:, W2_C + 128 * d: W2_C + 128 * (d + 1)] for d in range(3)]
            WF = [wts[:, WF_C + 8 * d: WF_C + 8 * (d + 1)] for d in range(3)]
            OSEL = wts[:, OSEL_C:OSEL_C + 32]
            SELB = wts[0:32, SELB_C:SELB_C + 128]
            ID32 = wts[0:32, ID32_C:ID32_C + 32]
            ONE1 = wts[0:1, ONE1_C:ONE1_C + 1]
            g32 = [cf[0:32, 0:1], cf[0:32, 2:3]]
            b32 = [cf[0:32, 1:2], cf[0:32, 3:4]]
            zb128 = cf[:, 5:6]
            zb1 = cf[0:1, 5:6]
            epsb = cf[0:32, 6:7]

            stkP = stk[:, 0]   # [128, 8, 512] views
            stkR = stk[:, 1]
            stkM = stk[:, 2]
            stkH = stk[:, 3]

            # ---- state ----
            X2 = stp.tile([32, 2, 2048], f32r, tag="X2")     # 0: x, 1: c1*rc
            x_t = X2[:, 0, :]
            rc1_t = X2[:, 1, :]
            S = stp.tile([32, 8, 512], f32r, tag="S")
            y_t, uy_t, arc_t = S[:, 0, :], S[:, 1, :], S[:, 2, :]
            h_t, r0_t, hu_t = S[:, 3, :], S[:, 4, :], S[:, 5, :]
            zmu_t, dif_t = S[:, 6, :], S[:, 7, :]
            s_t = stp.tile([32, 512], f32r, tag="sv")
            sq_t = stp.tile([32, 512], f32r, tag="sq")
            s32f = stp.tile([32, 1], f32, tag="s32f")
            nc.sync.dma_start(y_t[:], Y_d[:])
            nc.vector.tensor_scalar_mul(uy_t[:], y_t[:], -1.0)  # uy = -y

            zmuT = cmb.tile([128, 8, 32], bf, tag="zmuT")   # top 0-3, Nbot 4-7
            arcT = cmb.tile([128, 8, 32], bf, tag="arcT")   # top 0-3, Nbot 4-7
            sT = cmb.tile([128, 8, 32], bf, tag="sT")       # top 0-3, Hbot 4-7
            rcT = cmb.tile([128, 32, 32], bf, tag="rcT")    # top 0-15, Nbot 16-31
            smal = cmb.tile([32, 8], f32r, tag="smal")
            gb_t = smal[:, 1:3]
            mean_t, var_t = smal[:, 3:4], smal[:, 4:5]
            ssn_t, m2_t = smal[:, 5:6], smal[:, 6:7]
            row1 = cmb.tile([1, 128], f32, tag="row1")
            gbb = cmb.tile([128, 2], f32, tag="gbb")
            stat = cmb.tile([128, 4, 8], f32, tag="stat")   # [. bp, lt|4+lt]
            stat2 = cmb.tile([128, 2], f32, tag="stat2")
            stat2r = cmb.tile([128, 2], f32r, tag="stat2r")

            def combo_build(dst, src_bm, nchunk, kinds, sgn=1.0):
                """dst [128, k, 32] (bf16) = combo of sgn*src_bm [32, nchunk*128].
                chunks 0..nchunk-1: top [s_r|s_i]; then for 8-chunk layouts one
                of: 'N' bot [-s_i|s_r] or 'H' bot [s_i|-s_r] at nchunk..2nchunk-1.
                For rcT (nchunk=16): N-bot at 16..31."""
                for c in range(nchunk):
                    pT = psD.tile([128, 32], f32r, tag="tp")
                    nc.tensor.transpose(pT[:], src_bm[:, 128 * c:128 * (c + 1)],
                                        ID32)
                    nc.vector.tensor_scalar_mul(dst[:, c, :], pT[:], sgn)
                    o = nchunk
                    if 'H' in kinds:
                        nc.vector.tensor_scalar_mul(dst[:, o + c, 0:16],
                                                    pT[:, 16:32], sgn)
                        nc.vector.tensor_scalar_mul(dst[:, o + c, 16:32],
                                                    pT[:, 0:16], -sgn)
                    if 'N' in kinds:
                        nc.vector.tensor_scalar_mul(dst[:, o + c, 0:16],
                                                    pT[:, 16:32], -sgn)
                        nc.vector.tensor_scalar_mul(dst[:, o + c, 16:32],
                                                    pT[:, 0:16], sgn)

            def mv_nv(ps, gbase, lhsT_tile, rhs):
                """ps[32*gbase:+32, :512] = complex matvec over 8 chunks
                (lhsT_tile chunks 0..7) against rhs [128, 8, 512]."""
                for k in range(8):
                    nc.tensor.matmul(ps[32 * gbase:32 * gbase + 32, :],
                                     lhsT_tile[:, k, :], rhs[:, k, :],
                                     start=(k == 0), stop=(k == 7))

            def ah_apply(ps, lhsT_tile):
                """ps [128, 512]: col group g = nt tile g of A^H-type matvec.
                lhsT chunks 0..7 (top+Hbot)."""
                for k in range(8):
                    for g in range(4):
                        nc.tensor.matmul(
                            ps[32 * g:32 * g + 32, :], lhsT_tile[:, k, :],
                            ab[:, k, 512 * g:512 * (g + 1)],
                            start=(k == 0), stop=(k == 7),
                            tile_position=(0, 32 * g))

            # ---- x0 = A^H y ----
            p0 = psA.tile([128, 512], f32, tag="mm")
            ah_apply(p0, yl)
            for g in range(4):
                nc.scalar.copy(x_t[:, 512 * g:512 * (g + 1)],
                               p0[32 * g:32 * g + 32, :])

            # ================= iterations =================
            for it in range(ITERS):
                rho = float(rhos[it])
                eps = float(epss[it])
                c1 = 1.0 / (rho + 1e-8)
                last = (it == ITERS - 1)
                fbc1 = cf[0:32, 8 + it:9 + it]
                if nu > 1:
                    nc.sync.dma_start(stk[:], STK_d[prep['iter_stk_idx'][it]])

                # ---------- CNN ----------
                act1 = []
                for bp in range(4):
                    xin = xinp.tile([32, 2048], f32r, tag="xin")
                    nc.vector.memset(xin[:, 0:1].bitcast(f32), 0.0)
                    nc.vector.memset(xin[:, 2047:2048].bitcast(f32), 0.0)
                    for dl in range(3):
                        lo, hi = max(0, 1 - dl), min(2048, 2048 + 1 - dl)
                        for ci in range(2):
                            src = x_t[ci * 16 + bp * 4: ci * 16 + bp * 4 + 4,
                                      lo + dl - 1: hi + dl - 1]
                            nc.sync.dma_start(
                                xin[dl * 8 + ci * 4: dl * 8 + ci * 4 + 4, lo:hi], src)
                    a1 = actp.tile([128, 2050], f32r, tag="act")
                    nc.vector.memset(a1[:, 0:1].bitcast(f32), 0.0)
                    nc.vector.memset(a1[:, 2049:2050].bitcast(f32), 0.0)
                    for lt in range(4):
                        p = psB.tile([128, 512], f32, tag="big")
                        nc.tensor.matmul(p[:], W1[0:24, :],
                                         xin[0:24, 512 * lt:512 * (lt + 1)],
                                         start=True, stop=True)
                        # copy + running sum on scalar; square + sumsq on DVE
                        nc.scalar.activation(
                            a1[:, 1 + 512 * lt:1 + 512 * (lt + 1)], p[:],
                            AF.Identity, bias=zb128[:],
                            accum_out=stat[:, bp, lt:lt + 1])
                        sj = scp.tile([128, 512], f32r, tag="sqj")
                        asl = a1[:, 1 + 512 * lt:1 + 512 * (lt + 1)]
                        nc.vector.scalar_tensor_tensor(
                            sj[:], asl, 1.0, asl, OP.mult, OP.mult,
                            accum_out=stat[:, bp, 4 + lt:5 + lt])
                    act1.append(a1)

                def bn_apply(layer, acts, conv_next):
                    """Compute global BN affine from stat sums, then per-bp:
                    affine+relu (split scalar/DVE) and immediately the next
                    conv's matmuls for that bp (emitted by conv_next(bp))."""
                    with nc.allow_low_precision(reason="f32r rounding of fp32 sums"):
                        nc.vector.tensor_reduce(stat2[:, 0:1], stat[:, :, 0:4],
                                                AX.XY, OP.add)
                        nc.vector.tensor_reduce(stat2[:, 1:2], stat[:, :, 4:8],
                                                AX.XY, OP.add)
                    ci_ = ddr.tile([128, 2], f32, tag="cc")
                    co_ = ddr.tile([128, 2], f32, tag="cc")
                    nc.sync.dma_start(ci_[:], stat2[:])
                    nc.gpsimd.collective_compute(
                        "AllReduce", OP.add, replica_groups=[list(range(NCORE))],
                        ins=[ci_.opt()], outs=[co_.opt()])
                    nc.sync.dma_start(stat2[:], co_[:])
                    nc.vector.tensor_copy(stat2r[:], stat2[:])
                    p = psC.tile([32, 2], f32, tag="sm")
                    nc.tensor.matmul(p[:], OSEL, stat2r[:],
                                     start=True, stop=True)
                    with nc.allow_low_precision(reason="bn scalar math in f32r"):
                        nc.vector.tensor_scalar_mul(mean_t[:], p[:, 0:1], 1.0 / cnt)
                        nc.vector.tensor_scalar_mul(ssn_t[:], p[:, 1:2], 1.0 / cnt)
                        nc.vector.tensor_mul(m2_t[:], mean_t[:], mean_t[:])
                        nc.vector.tensor_sub(var_t[:], ssn_t[:], m2_t[:])
                        nc.scalar.activation(var_t[:], var_t[:], AF.Sqrt,
                                             bias=epsb[:])
                        nc.vector.reciprocal(var_t[:], var_t[:])
                        nc.vector.tensor_mul(gb_t[:, 0:1], g32[layer][:], var_t[:])
                        nc.vector.tensor_mul(m2_t[:], mean_t[:], gb_t[:, 0:1])
                        nc.vector.tensor_sub(gb_t[:, 1:2], b32[layer][:], m2_t[:])
                    p2 = psC.tile([128, 2], f32, tag="sm")
                    nc.tensor.matmul(p2[:], SELB, gb_t[:],
                                     start=True, stop=True)
                    nc.vector.tensor_copy(gbb[:], p2[:])
                    SP = 1280   # relu split: scalar engine cols [1:1+SP), DVE rest
                    for bp, a in enumerate(acts):
                        nc.scalar.activation(a[:, 1:1 + SP], a[:, 1:1 + SP],
                                             AF.Relu, bias=gbb[:, 1:2],
                                             scale=gbb[:, 0:1])
                        nc.vector.tensor_scalar(a[:, 1 + SP:2049],
                                                a[:, 1 + SP:2049],
                                                gbb[:, 0:1], gbb[:, 1:2],
                                                OP.mult, OP.add)
                        nc.vector.tensor_scalar_max(a[:, 1 + SP:2049],
                                                    a[:, 1 + SP:2049], 0.0)
                        conv_next(bp)

                # conv2 (emitted per-bp right after that bp's relu)
                act2 = []

                def conv2_bp(bp):
                    a2 = actp.tile([128, 2050], f32r, tag="act")
                    nc.vector.memset(a2[:, 0:1].bitcast(f32), 0.0)
                    nc.vector.memset(a2[:, 2049:2050].bitcast(f32), 0.0)
                    for lt in range(4):
                        p = psB.tile([128, 512], f32, tag="big")
                        for dl in range(3):
                            nc.tensor.matmul(
                                p[:], W2[dl],
                                act1[bp][:, dl + 512 * lt: dl + 512 * lt + 512],
                                start=(dl == 0), stop=(dl == 2))
                        nc.scalar.activation(
                            a2[:, 1 + 512 * lt:1 + 512 * (lt + 1)], p[:],
                            AF.Identity, bias=zb128[:],
                            accum_out=stat[:, bp, lt:lt + 1])
                        sj = scp.tile([128, 512], f32r, tag="sqj")
                        asl = a2[:, 1 + 512 * lt:1 + 512 * (lt + 1)]
                        nc.vector.scalar_tensor_tensor(
                            sj[:], asl, 1.0, asl, OP.mult, OP.mult,
                            accum_out=stat[:, bp, 4 + lt:5 + lt])
                    act2.append(a2)

                bn_apply(0, act1, conv2_bp)

                # convf + residual (per-bp after that bp's relu): col-tiled
                # 3-dl concurrent MMs, then 2-level sum, then scatter into rc1
                def convf_bp(bp):
                    for lt in range(4):
                        p = psB.tile([128, 512], f32, tag="big")
                        for dl in range(3):
                            nc.tensor.matmul(
                                p[0:8, :], WF[dl],
                                act2[bp][:, dl + 512 * lt: dl + 512 * lt + 512],
                                start=(dl == 0), stop=(dl == 2))
                        s8 = scp.tile([8, 512], f32r, tag="s8")
                        nc.scalar.copy(s8[:], p[0:8, :])
                        for cfi in range(2):
                            nc.sync.dma_start(
                                rc1_t[cfi * 16 + bp * 4: cfi * 16 + bp * 4 + 4,
                                      512 * lt:512 * (lt + 1)],
                                s8[cfi * 4:cfi * 4 + 4, :])

                bn_apply(1, act2, convf_bp)

                # rc1 = (rc1_raw + x)*c1 + fb*c1, per nt chunk, then transposes
                for nt in range(4):
                    sl = slice(512 * nt, 512 * (nt + 1))
                    nc.vector.tensor_add(rc1_t[:, sl], rc1_t[:, sl], x_t[:, sl])
                    nc.scalar.activation(rc1_t[:, sl], rc1_t[:, sl],
                                         AF.Identity, bias=fbc1, scale=c1)
                    for c in range(4 * nt, 4 * nt + 4):
                        pT = psD.tile([128, 32], f32r, tag="tp")
                        nc.tensor.transpose(
                            pT[:], rc1_t[:, 128 * c:128 * (c + 1)], ID32)
                        nc.vector.tensor_copy(rcT[:, c, :], pT[:])
                        nc.vector.tensor_scalar_mul(rcT[:, 16 + c, 0:16],
                                                    pT[:, 16:32], -1.0)
                        nc.vector.tensor_copy(rcT[:, 16 + c, 16:32],
                                              pT[:, 0:16])

                # arc = A rc : 32 chunks split into 4 col groups + 3 adds
                parc = psA.tile([128, 512], f32, tag="mm")
                for k in range(8):
                    for g in range(4):
                        kc = 8 * g + k
                        nc.tensor.matmul(parc[32 * g:32 * g + 32, :],
                                         rcT[:, kc, :], at[:, kc, :],
                                         start=(k == 0), stop=(k == 7),
                                         tile_position=(0, 32 * g))
                nc.scalar.copy(sq_t[:], parc[0:32, :])
                nc.vector.scalar_tensor_tensor(sq_t[:], parc[32:64, :], 1.0,
                                               sq_t[:], OP.mult, OP.add)
                nc.scalar.copy(s_t[:], parc[64:96, :])
                nc.vector.scalar_tensor_tensor(s_t[:], parc[96:128, :], 1.0,
                                               s_t[:], OP.mult, OP.add)
                nc.vector.tensor_add(arc_t[:], sq_t[:], s_t[:])

                # h = Hm arc (col grp 1), r0 = Mn arc (col grp 0), hu = h + uy
                combo_build(arcT, arc_t, 4, {'N'})
                phr = psA.tile([128, 512], f32, tag="mm")
                mv_nv(phr, 0, arcT, stkM)
                mv_nv(phr, 1, arcT, stkH)
                nc.scalar.copy(r0_t[:], phr[0:32, :])
                nc.scalar.copy(h_t[:], phr[32:64, :])
                nc.vector.scalar_tensor_tensor(hu_t[:], phr[32:64, :], 1.0,
                                               uy_t[:], OP.mult, OP.add)

                # ---------- ADMM ----------
                for s in range(ADMM):
                    final = (s == ADMM - 1)
                    if s == 0:
                        combo_build(zmuT, uy_t, 4, {'N'}, sgn=-1.0)  # zmu=-uy
                    else:
                        combo_build(zmuT, zmu_t, 4, {'N'})
                    if final:
                        pds = psA.tile([128, 512], f32, tag="mm")
                        if not last:
                            mv_nv(pds, 0, zmuT, stkP)   # Ax-part for diff
                        mv_nv(pds, 1, zmuT, stkR)       # s-part
                        nc.vector.scalar_tensor_tensor(
                            s_t[:], pds[32:64, :], 1.0, r0_t[:],
                            OP.mult, OP.add)
                        combo_build(sT, s_t, 4, {'H'})
                        px = psA.tile([128, 512], f32, tag="mm")
                        ah_apply(px, sT)
                        for g in range(4):
                            sl = slice(512 * g, 512 * (g + 1))
                            nc.vector.scalar_tensor_tensor(
                                x_t[:, sl], px[32 * g:32 * g + 32, :], 1.0,
                                rc1_t[:, sl], OP.mult, OP.add)
                        if last:
                            continue
                        pdif = pds[0:32, :]
                    else:
                        pds = psA.tile([128, 512], f32, tag="mm")
                        mv_nv(pds, 0, zmuT, stkP)
                        pdif = pds[0:32, :]

                    # diff = P'zmu + h + uy ; n2 = per-sample |diff|^2
                    nc.vector.scalar_tensor_tensor(dif_t[:], pdif, 1.0,
                                                   hu_t[:], OP.mult, OP.add)
                    nc.vector.scalar_tensor_tensor(sq_t[:], dif_t[:], 1.0,
                                                   dif_t[:], OP.mult, OP.mult,
                                                   accum_out=s32f[:])
                    pt = psC.tile([1, 32], f32, tag="sm")
                    nc.tensor.matmul(pt[:], s32f[:], ID32.bitcast(f32),
                                     is_transpose=True)
                    nc.vector.tensor_copy(row1[:, 0:32], pt[:])
                    nc.vector.tensor_add(row1[:, 32:48], row1[:, 0:16],
                                         row1[:, 16:32])
                    # fac = min(1, eps/sqrt(n2)) = min(1, 1/sqrt(n2/eps^2))
                    nc.scalar.activation(row1[:, 48:64], row1[:, 32:48],
                                         AF.Sqrt, bias=zb1[:],
                                         scale=1.0 / (eps * eps))
                    nc.vector.reciprocal(row1[:, 48:64], row1[:, 48:64])
                    nc.vector.tensor_scalar_min(row1[:, 48:64],
                                                row1[:, 48:64], 1.0)
                    # t = fac-1 ; f2 = fac+t (cols 64:96 2x) ; f1 = -t (96:128)
                    nc.vector.tensor_scalar_add(row1[:, 96:112],
                                                row1[:, 48:64], -1.0)
                    nc.vector.tensor_add(row1[:, 64:80], row1[:, 48:64],
                                         row1[:, 96:112])
                    nc.vector.tensor_add(row1[:, 80:96], row1[:, 48:64],
                                         row1[:, 96:112])
                    nc.vector.tensor_scalar_mul(row1[:, 96:112],
                                                row1[:, 96:112], -1.0)
                    nc.vector.tensor_copy(row1[:, 112:128], row1[:, 96:112])
                    fr = psC.tile([64, 1], f32, tag="sm")
                    nc.tensor.matmul(fr[:], row1[:, 64:128],
                                     ONE1.bitcast(f32), is_transpose=True)
                    if not final:
                        nc.vector.scalar_tensor_tensor(
                            zmu_t[:], dif_t[:], fr[0:32, :], y_t[:],
                            OP.mult, OP.add)
                    nc.vector.scalar_tensor_tensor(
                        uy_t[:], dif_t[:], fr[32:64, :], y_t[:],
                        OP.mult, OP.subtract)
                    if not final:
                        nc.vector.scalar_tensor_tensor(
                            hu_t[:], h_t[:], 1.0, uy_t[:], OP.mult, OP.add)

            nc.sync.dma_start(XO_d[:], x_t[:])

    nc.compile()
    return nc


def _enable_trace_shim():
    import sys, types
    try:
        import trn_agent_boot.trn_boot as _tb
        import concourse.bass_utils as _bu
        _bu.upload_artifacts = lambda tmpdir: "local://" + str(tmpdir)
        hookmod = types.ModuleType('antenv.axon_hooks')
        hook = _tb._ntff_profile_via_ctypes('/opt/axon/libaxon_pjrt.so')
        hookmod.get_axon_ntff_profile_hook = lambda: hook
        import antenv as _antenv
        sys.modules['antenv.axon_hooks'] = hookmod
        _antenv.axon_hooks = hookmod
        return True
    except Exception:
        return False


def kernel(**inputs) -> np.ndarray:
    import os
    from concourse.bass_utils import run_bass_kernel_spmd
    trace = bool(os.environ.get("KERNEL_TRACE"))
    if trace:
        trace = _enable_trace_shim()

    prep = _host_prep(inputs)
    nc = _build_program(prep)

    stks = np.stack(prep['stk_list'], 0)
    in_maps = []
    for c in range(NCORE):
        in_maps.append({
            "AB": prep['AB'], "ATR": prep['ATR'], "STKS": stks,
            "WTS": prep['WTS'], "CF": prep['CF'],
            "YBM": np.ascontiguousarray(prep['ybm_cores'][c][:, :512]),
            "YL": prep['ylhsT_cores'][c],
        })
    res = run_bass_kernel_spmd(nc, in_maps, list(range(NCORE)), trace=trace)
    out = np.zeros((B, 2, Nt), np.float32)
    for c in range(NCORE):
        xc = res.results[c]["XOUT"]
        out[c * BS:(c + 1) * BS, 0] = xc[:16]
        out[c * BS:(c + 1) * BS, 1] = xc[16:]
    kernel._last_results = res
    return out
